# revision 1
# baseline (speedup 1.0000x reference)
"""3-layer GAT on 8 Trainium2 NeuronCores (Bass/Tile).

Strategy (graph/data parallel, edges partitioned by destination node):
 - Host: add self-loops, sort nodes by in-degree, renumber, group nodes into
   blocks of 128 with per-block uniform edge-slot counts (degree-equalized
   padding), deal blocks round-robin to the 8 cores so the per-position
   block schedule is identical across cores (one SPMD program).
 - Device, per layer (one launch per layer; host transposes between):
     phase 1 (replicated): table[n] = [x@W (bf16) | x@(W a_src) (fp32)] for
       all nodes, 512-byte rows, written to two overlapping tables (A covers
       low nodes, B high nodes) so dma_gather's int16 row indices suffice.
     phase 2 (own blocks): per 128-node dst block, two dma_gathers (table A
       slots + table B slots) fetch all edge rows; w = exp(leakyrelu(alpha_s
       + alpha_d)) with alpha_d a host-precomputed per-partition scalar;
       attention-weighted sums and softmax denominators are free-dim
       reductions; epilogue bias/residual/ELU; write own rows.
 - Segment softmax skips max-subtraction (logits are O(10); fp32 exp safe).
 - Each table has a DUMMY row (alpha_s=-1e30 -> w=0) for padded slots and a
   SAFE row (all 0 -> w=1) so zero-degree padding nodes keep finite
   denominators.
"""

import os
import sys

sys.path.insert(0, "/opt/trn_rl_repo")
import ml_dtypes
import numpy as np

import concourse.bass as bass
import concourse.bacc as bacc
import concourse.mybir as mybir
import concourse.tile as tile
from concourse.bass_utils import run_bass_kernel_spmd

# problem constants (hardcoded per spec nn_GAT_10041633538818)
F = 128
HH = 4  # heads
CC = 32  # channels/head, layers 1-2
NCLS = 40  # classes (head dim, layer 3)
NEG = 0.2  # leaky relu slope
P = 128
ROW = 256  # bf16 elems per table row (512 B)

f32 = mybir.dt.float32
bf16 = mybir.dt.bfloat16
i16 = mybir.dt.int16
u16 = mybir.dt.uint16

bfloat16 = ml_dtypes.bfloat16

LAST_EXEC_NS = None  # filled by kernel() when GAT_TRACE=1


# ----------------------------------------------------------------- host prep


def _make_geometry(n, n_cores):
    nblk = -(-n // P)
    nblk = -(-nblk // n_cores) * n_cores
    npad = nblk * P
    # table split: A covers blocks [0, na_blk), B covers [nblk-na_blk, nblk);
    # both tables hold na_blk*128 (+2 special) rows, int16-indexable.
    grp = 4
    na_blk = min(nblk, 252)  # 252*128 + 2 = 32258 <= 32767
    na_blk -= na_blk % grp
    b0_blk = nblk - na_blk
    assert b0_blk % grp == 0 and na_blk % grp == 0
    namax = na_blk * P
    return dict(
        n=n,
        n_cores=n_cores,
        nblk=nblk,
        npad=npad,
        bpc=nblk // n_cores,
        grp=grp,
        na_blk=na_blk,
        b0_blk=b0_blk,
        b0=b0_blk * P,
        namax=namax,  # nodes per table
        dummy=namax,  # special rows (same index in both tables)
        safe=namax + 1,
        trows=namax + 2,
    )


def _pack16(flat):
    """flat [n] idx -> [128, n/16] int16 (16-partition wrap, replicated 8x)"""
    a = np.asarray(flat, np.int16).reshape(-1, 16).T
    return np.tile(a, (8, 1))


def _prep_graph(geom, edge_index):
    """Returns (order, Ma, Mb, idx, cofs) — degree-sort order, per-position
    A/B slot schedules, per-core packed int16 gather indices [ncores, 128, C],
    and per-position (A, B) column offsets into that array."""
    n = geom["n"]
    npad = geom["npad"]
    nblk = geom["nblk"]
    ncores = geom["n_cores"]
    bpc = geom["bpc"]
    b0 = geom["b0"]
    namax = geom["namax"]

    loops = np.arange(n, dtype=np.int64)
    src = np.concatenate([edge_index[0].astype(np.int64), loops])
    dst = np.concatenate([edge_index[1].astype(np.int64), loops])

    deg = np.bincount(dst, minlength=n)
    order = np.argsort(deg, kind="stable")
    rank = np.empty(n, np.int64)
    rank[order] = np.arange(n)
    srcs = rank[src]
    dsts = rank[dst]

    degs = np.zeros(npad, np.int64)
    degs[:n] = deg[order]
    starts = np.zeros(npad + 1, np.int64)
    starts[1:] = np.cumsum(degs)

    # split each node's edges between tables A and B, balancing the two
    must_a = srcs < b0  # below B's range
    must_b = srcs >= namax  # above A's range
    a_only = np.bincount(dsts[must_a], minlength=npad)
    b_only = np.bincount(dsts[must_b], minlength=npad)
    half = (degs + 1) // 2
    flex = degs - a_only - b_only
    x = np.clip(half - a_only, 0, flex)
    need_a = a_only + x
    need_b = degs - need_a

    key = np.where(must_a, 0, np.where(must_b, 2, 1))
    eord = np.lexsort((key, dsts))
    srcs2 = srcs[eord]
    dsts2 = dsts[eord]
    pos = np.arange(len(srcs2)) - starts[dsts2]
    in_a = pos < need_a[dsts2]

    ma_max = int(need_a.max()) if len(need_a) else 1
    mb_max = int(need_b.max()) if len(need_b) else 1
    Aarr = np.full((npad, max(ma_max, 1)), geom["dummy"], np.int64)
    Barr = np.full((npad, max(mb_max, 1)), geom["dummy"], np.int64)
    Aarr[dsts2[in_a], pos[in_a]] = srcs2[in_a]
    nb_pos = (pos - need_a[dsts2])[~in_a]
    Barr[dsts2[~in_a], nb_pos] = srcs2[~in_a] - b0
    # zero-degree padding nodes: one SAFE slot in table A
    zdeg = np.nonzero(degs == 0)[0]
    Aarr[zdeg, 0] = geom["safe"]
    need_a[zdeg] = 1

    na_blkmax = need_a.reshape(nblk, P).max(axis=1)
    nb_blkmax = need_b.reshape(nblk, P).max(axis=1)
    Ma = [int(na_blkmax[ncores * j : ncores * (j + 1)].max()) for j in range(bpc)]
    Mb = [int(nb_blkmax[ncores * j : ncores * (j + 1)].max()) for j in range(bpc)]

    # per-core packed int16 index arrays
    cidx = 8 * (sum(Ma) + sum(Mb))
    idx = np.empty((ncores, P, cidx), np.int16)
    cofs = []
    off = 0
    for j in range(bpc):
        ofa, ofb = off, off + 8 * Ma[j]
        cofs.append((ofa, ofb))
        off = ofb + 8 * Mb[j]
        for k in range(ncores):
            b = ncores * j + k
            rows = slice(b * P, (b + 1) * P)
            if Ma[j]:
                flat = Aarr[rows, : Ma[j]].T.reshape(-1)  # slot-major
                idx[k, :, ofa : ofa + 8 * Ma[j]] = _pack16(flat)
            if Mb[j]:
                flat = Barr[rows, : Mb[j]].T.reshape(-1)
                idx[k, :, ofb : ofb + 8 * Mb[j]] = _pack16(flat)
    return order, Ma, Mb, idx, cofs


def _pack_blocks(geom, arr, k):
    w = arr.shape[-1]
    blocks = arr.reshape(geom["nblk"], P, w)[k :: geom["n_cores"]]
    return np.ascontiguousarray(blocks.transpose(1, 0, 2).reshape(P, -1))


def _pack_rows(geom, arr, k):
    w = arr.shape[-1]
    blocks = arr.reshape(geom["nblk"], P, w)[k :: geom["n_cores"]]
    return np.ascontiguousarray(blocks.reshape(-1, w))


def _unpack_rows(geom, outs):
    w = outs[0].shape[-1]
    full = np.empty((geom["npad"], w), np.float32)
    blocks = full.reshape(geom["nblk"], P, w)
    for k in range(geom["n_cores"]):
        blocks[k :: geom["n_cores"]] = outs[k].reshape(geom["bpc"], P, w)
    return full


def _combine_w(W, a):
    h, c = a.shape
    wa = np.einsum("fhc,hc->fh", W.reshape(W.shape[0], h, c), a)
    return np.ascontiguousarray(np.concatenate([W, wa], axis=1).astype(np.float32))


# ------------------------------------------------------------ device program


def _build_program(geom, Ma, Mb, cofs, dout, outc, layer3):
    T = dout + HH  # matmul output cols (h | alpha_s)
    bpc = geom["bpc"]
    npad = geom["npad"]
    trows = geom["trows"]
    chead = dout // HH
    cidx = 8 * (sum(Ma) + sum(Mb))
    GRP = geom["grp"]
    na_blk = geom["na_blk"]
    b0_blk = geom["b0_blk"]
    ahalf = dout // 2  # fp32 column where alpha_s lives inside a row

    nc = bacc.Bacc(
        "TRN2",
        target_bir_lowering=False,
        debug=False,
        enable_asserts=False,
        num_devices=geom["n_cores"],
    )
    xT = nc.declare_dram_parameter("xT", [P, npad], f32, isOutput=False)
    wc = nc.declare_dram_parameter("wc", [P, T], f32, isOutput=False)
    biasp = nc.declare_dram_parameter("bias", [P, outc], f32, isOutput=False)
    idxp = nc.declare_dram_parameter("idx", [P, cidx], i16, isOutput=False)
    adp = nc.declare_dram_parameter("ad", [P, bpc * HH], f32, isOutput=False)
    if not layer3:
        resp = nc.declare_dram_parameter("res", [bpc * P, outc], f32, isOutput=False)
    xout = nc.declare_dram_parameter("xout", [bpc * P, outc], f32, isOutput=True)
    tabA = nc.dram_tensor("tabA", [trows, ROW], u16)
    tabB = nc.dram_tensor("tabB", [trows, ROW], u16)

    Exp = mybir.ActivationFunctionType.Exp
    Copy = mybir.ActivationFunctionType.Copy
    ADD = mybir.AluOpType.add
    MIN = mybir.AluOpType.min
    MAX = mybir.AluOpType.max
    MULT = mybir.AluOpType.mult
    AX = mybir.AxisListType.X

    with tile.TileContext(nc) as tc:
        with (
            tc.tile_pool(name="const", bufs=1) as cp,
            tc.tile_pool(name="xb", bufs=3) as xbp,
            tc.tile_pool(name="mm", bufs=4, space="PSUM") as psp,
            tc.tile_pool(name="st", bufs=3) as stp,
            tc.tile_pool(name="edge", bufs=2) as ep,
            tc.tile_pool(name="small", bufs=3) as sp,
        ):
            wc_t = cp.tile([P, T], f32)
            nc.sync.dma_start(wc_t[:], wc[:])
            bias_t = cp.tile([P, outc], f32)
            nc.sync.dma_start(bias_t[:], biasp[:])
            idx_t = cp.tile([P, cidx], i16)
            nc.sync.dma_start(idx_t[:], idxp[:])
            ad_t = cp.tile([P, bpc * HH], f32)
            nc.sync.dma_start(ad_t[:], adp[:])

            # special rows: DUMMY (alpha_s=-1e30 -> w=0), SAFE (zeros -> w=1)
            spec = cp.tile([2, ROW], u16)
            nc.vector.memset(spec[:], 0.0)
            nc.vector.memset(spec[:].bitcast(f32)[0:1, ahalf : ahalf + HH], -1.0e30)
            nc.sync.dma_start(tabA[geom["dummy"] : geom["dummy"] + 2, :], spec[:])
            nc.sync.dma_start(tabB[geom["dummy"] : geom["dummy"] + 2, :], spec[:])

            # ---- phase 1 (replicated): tables = [x@W (bf16) | x@(W a) (f32)]
            CH = 2048
            nblk = geom["nblk"]
            gstage = None
            for c0 in range(0, npad, CH):
                wlen = min(CH, npad - c0)
                xbuf = xbp.tile([P, wlen], f32, tag="xbuf")
                nc.sync.dma_start(xbuf[:], xT[:, c0 : c0 + wlen])
                for b in range(wlen // P):
                    gb = c0 // P + b
                    ps = psp.tile([P, T], f32, tag="mmps")
                    nc.tensor.matmul(
                        out=ps[:],
                        lhsT=xbuf[:, b * P : (b + 1) * P],
                        rhs=wc_t[:],
                        start=True,
                        stop=True,
                    )
                    si = gb % GRP
                    if si == 0:
                        gstage = stp.tile([P, GRP * ROW], u16, tag="gstage")
                        # zero the per-row pad region (bytes never computed)
                        nc.vector.memset(
                            gstage[:]
                            .rearrange("p (c f) -> p c f", c=GRP)[
                                :, :, dout + 2 * HH : ROW
                            ],
                            0.0,
                        )
                    nc.scalar.copy(
                        gstage[:].bitcast(bf16)[:, si * ROW : si * ROW + dout],
                        ps[:, 0:dout],
                    )
                    nc.vector.tensor_copy(
                        gstage[:].bitcast(f32)[
                            :, si * (ROW // 2) + ahalf : si * (ROW // 2) + ahalf + HH
                        ],
                        ps[:, dout:T],
                    )
                    if si == GRP - 1:
                        g0 = gb - GRP + 1
                        sview = gstage[:].rearrange("p (c f) -> p c f", c=GRP)
                        if gb < na_blk:
                            drows = tabA[g0 * P : (gb + 1) * P, :].rearrange(
                                "(c p) f -> p c f", p=P
                            )
                            nc.sync.dma_start(drows, sview)
                        if g0 >= b0_blk:
                            r0 = (g0 - b0_blk) * P
                            drows = tabB[r0 : r0 + GRP * P, :].rearrange(
                                "(c p) f -> p c f", p=P
                            )
                            nc.sync.dma_start(drows, sview)

            # ---- phase 2: per own dst block
            dbg = os.environ.get("GAT_DBG", "full")
            for j in range(bpc):
                ma, mb = Ma[j], Mb[j]
                m = ma + mb
                ofa, ofb = cofs[j]
                if dbg == "phase1":
                    xo = sp.tile([P, outc], f32, tag="xo")
                    nc.vector.memset(xo[:], 0.0)
                    nc.sync.dma_start(xout[j * P : (j + 1) * P, :], xo[:])
                    continue
                g = ep.tile([P, m * ROW], u16, tag="g")
                g3 = g[:].bitcast(bf16).rearrange("p (m t) -> p m t", m=m)
                gu3 = g[:].rearrange("p (m t) -> p m t", m=m)
                CAP = 8  # dma_gather HW limit: num_idxs <= 1024 = 8*P
                for s0 in range(0, ma, CAP):
                    s1 = min(s0 + CAP, ma)
                    nc.gpsimd.dma_gather(
                        gu3[:, s0:s1, :],
                        tabA[:],
                        idx_t[:, ofa + 8 * s0 : ofa + 8 * s1],
                        P * (s1 - s0), P * (s1 - s0), ROW,
                    )
                for s0 in range(0, mb, CAP):
                    s1 = min(s0 + CAP, mb)
                    nc.gpsimd.dma_gather(
                        gu3[:, ma + s0 : ma + s1, :],
                        tabB[:],
                        idx_t[:, ofb + 8 * s0 : ofb + 8 * s1],
                        P * (s1 - s0), P * (s1 - s0), ROW,
                    )
                if dbg == "gather":
                    # consume g minimally to force the dependency
                    xo = sp.tile([P, outc], f32, tag="xo")
                    nc.vector.reduce_sum(
                        out=xo[:, 0:1],
                        in_=g[:].bitcast(bf16)[:, 0 : m * ROW],
                        axis=mybir.AxisListType.X,
                    )
                    nc.vector.memset(xo[:, 1:outc], 0.0)
                    nc.sync.dma_start(xout[j * P : (j + 1) * P, :], xo[:])
                    continue
                gf3 = g[:].bitcast(f32).rearrange("p (m t) -> p m t", m=m)

                # w = exp(leakyrelu(alpha_s + alpha_d))
                w_t = sp.tile([P, m * HH], f32, tag="w")
                w3 = w_t[:].rearrange("p (m h) -> p m h", m=m)
                u_t = sp.tile([P, m * HH], f32, tag="u")
                u3 = u_t[:].rearrange("p (m h) -> p m h", m=m)
                ad_b = (
                    ad_t[:, j * HH : (j + 1) * HH].unsqueeze(1).to_broadcast([P, m, HH])
                )
                nc.vector.tensor_tensor(
                    out=w3, in0=gf3[:, :, ahalf : ahalf + HH], in1=ad_b, op=ADD
                )
                nc.vector.tensor_scalar(
                    out=u3, in0=w3, scalar1=0.0, scalar2=-(1.0 - NEG),
                    op0=MIN, op1=MULT,
                )
                nc.vector.tensor_tensor(out=w3, in0=w3, in1=u3, op=ADD)
                nc.scalar.activation(out=w_t[:], in_=w_t[:], func=Exp)
                wb_t = sp.tile([P, m * HH], bf16, tag="wb")
                nc.vector.tensor_copy(wb_t[:], w_t[:])
                wb3 = wb_t[:].rearrange("p (m h) -> p m h", m=m)

                # messages (in place, bf16) and reductions
                gh = g3[:, :, 0:dout].rearrange("p m (h c) -> p m h c", h=HH)
                wb = wb3.unsqueeze(3).to_broadcast([P, m, HH, chead])
                nc.vector.tensor_tensor(out=gh, in0=gh, in1=wb, op=MULT)

                acc = sp.tile([P, dout], f32, tag="acc")
                nc.vector.reduce_sum(
                    out=acc[:], in_=g3[:, :, 0:dout].transpose([0, 2, 1]), axis=AX
                )
                den = sp.tile([P, HH], f32, tag="den")
                nc.vector.reduce_sum(out=den[:], in_=wb3.transpose([0, 2, 1]), axis=AX)
                rec = sp.tile([P, HH], f32, tag="rec")
                nc.vector.reciprocal(rec[:], den[:])

                xo = sp.tile([P, outc], f32, tag="xo")
                if not layer3:
                    for h in range(HH):
                        nc.scalar.activation(
                            out=xo[:, h * chead : (h + 1) * chead],
                            in_=acc[:, h * chead : (h + 1) * chead],
                            func=Copy,
                            scale=rec[:, h : h + 1],
                        )
                    res_t = sp.tile([P, outc], f32, tag="res")
                    nc.sync.dma_start(res_t[:], resp[j * P : (j + 1) * P, :])
                    nc.vector.tensor_tensor(out=xo[:], in0=xo[:], in1=bias_t[:], op=ADD)
                    nc.vector.tensor_tensor(out=xo[:], in0=xo[:], in1=res_t[:], op=ADD)
                    # elu: xo = (max(xo,0) - 1) + exp(min(xo,0))
                    tt = sp.tile([P, outc], f32, tag="tt")
                    nc.vector.tensor_scalar(
                        out=tt[:], in0=xo[:], scalar1=0.0, scalar2=None, op0=MIN
                    )
                    nc.scalar.activation(out=tt[:], in_=tt[:], func=Exp)
                    nc.vector.tensor_scalar(
                        out=xo[:], in0=xo[:], scalar1=0.0, scalar2=-1.0,
                        op0=MAX, op1=ADD,
                    )
                    nc.vector.tensor_tensor(out=xo[:], in0=xo[:], in1=tt[:], op=ADD)
                else:
                    tmp = sp.tile([P, dout], f32, tag="t3")
                    for h in range(HH):
                        nc.scalar.activation(
                            out=tmp[:, h * chead : (h + 1) * chead],
                            in_=acc[:, h * chead : (h + 1) * chead],
                            func=Copy,
                            scale=rec[:, h : h + 1],
                        )
                    t4 = tmp[:].rearrange("p (h c) -> p h c", h=HH)
                    nc.vector.tensor_tensor(
                        out=t4[:, 0, :], in0=t4[:, 0, :], in1=t4[:, 1, :], op=ADD
                    )
                    nc.vector.tensor_tensor(
                        out=t4[:, 2, :], in0=t4[:, 2, :], in1=t4[:, 3, :], op=ADD
                    )
                    nc.vector.tensor_tensor(
                        out=xo[:], in0=t4[:, 0, :], in1=t4[:, 2, :], op=ADD
                    )
                    nc.vector.tensor_scalar(
                        out=xo[:], in0=xo[:], scalar1=0.25, scalar2=None, op0=MULT
                    )
                    nc.vector.tensor_tensor(out=xo[:], in0=xo[:], in1=bias_t[:], op=ADD)
                nc.sync.dma_start(xout[j * P : (j + 1) * P, :], xo[:])
    return nc


# ------------------------------------------------------------------ numpy ref


def _emulate_launch(geom, Ma, Mb, xT_arr, wc_arr, bias_arr, idxs, cofs, ads, ress,
                    dout, outc, layer3):
    """numpy emulation (bf16-rounded where the device rounds)."""
    T = dout + HH
    chead = dout // HH
    npad = geom["npad"]
    b0 = geom["b0"]
    namax = geom["namax"]
    hfull = (xT_arr.T @ wc_arr).astype(np.float32)
    h16 = hfull[:, :dout].astype(bfloat16)
    als = hfull[:, dout:T]

    def row(tab_is_b, ridx):
        out_h = np.zeros((len(ridx), dout), np.float32)
        out_a = np.zeros((len(ridx), HH), np.float32)
        base = b0 if tab_is_b else 0
        node = np.asarray(ridx, np.int64) + base
        real = np.asarray(ridx) < namax
        out_h[real] = h16[node[real]].astype(np.float32)
        out_a[real] = als[node[real]]
        out_a[np.asarray(ridx) == geom["dummy"]] = -1.0e30
        return out_h, out_a

    outs = []
    for k in range(geom["n_cores"]):
        ad = ads[k]
        rows_out = []
        for j in range(geom["bpc"]):
            ma, mb = Ma[j], Mb[j]
            m = ma + mb
            ofa, ofb = cofs[j]
            # unpack per-slot indices back to [P, m]
            gi = np.empty((P, m), np.int64)
            if ma:
                flat = (
                    idxs[k][:16, ofa : ofa + 8 * ma].T.reshape(-1).astype(np.int64)
                )
                gi[:, :ma] = flat.reshape(ma, P).T
            if mb:
                flat = (
                    idxs[k][:16, ofb : ofb + 8 * mb].T.reshape(-1).astype(np.int64)
                )
                gi[:, ma:] = flat.reshape(mb, P).T
            gh = np.empty((P, m, dout), np.float32)
            ga = np.empty((P, m, HH), np.float32)
            if ma:
                hh, aa = row(False, gi[:, :ma].reshape(-1))
                gh[:, :ma] = hh.reshape(P, ma, dout)
                ga[:, :ma] = aa.reshape(P, ma, HH)
            if mb:
                hh, aa = row(True, gi[:, ma:].reshape(-1))
                gh[:, ma:] = hh.reshape(P, mb, dout)
                ga[:, ma:] = aa.reshape(P, mb, HH)
            e = ga + ad[:, j * HH : (j + 1) * HH][:, None, :]
            lre = np.where(e > 0, e, NEG * e).astype(np.float32)
            w = np.exp(lre)
            w16 = w.astype(bfloat16)
            h4 = gh.reshape(P, m, HH, chead)
            msg = (h4 * w16.astype(np.float32)[..., None]).astype(bfloat16)
            accv = msg.astype(np.float32).sum(axis=1)
            den = w16.astype(np.float32).sum(axis=1)
            r = accv / den[..., None]
            if layer3:
                xo = r.mean(axis=1) + bias_arr[0]
            else:
                xo = r.reshape(P, dout) + bias_arr[0] + ress[k][j * P : (j + 1) * P]
                xo = np.where(xo > 0, xo, np.expm1(np.minimum(xo, 0)))
            rows_out.append(xo.astype(np.float32))
        outs.append(np.concatenate(rows_out, axis=0))
    return outs


# ---------------------------------------------------------------------- main


def kernel(**inputs):
    global LAST_EXEC_NS
    x = np.asarray(inputs["x"], np.float32)
    edge_index = np.asarray(inputs["edge_index"], np.int32)
    Ws = [np.asarray(inputs[f"W{i}"], np.float32) for i in (1, 2, 3)]
    asrc = [np.asarray(inputs[f"a_src{i}"], np.float32) for i in (1, 2, 3)]
    adst = [np.asarray(inputs[f"a_dst{i}"], np.float32) for i in (1, 2, 3)]
    bs = [np.asarray(inputs[f"b{i}"], np.float32) for i in (1, 2, 3)]

    n = x.shape[0]
    ncores = 8
    geom = _make_geometry(n, ncores)
    order, Ma, Mb, idx, cofs = _prep_graph(geom, edge_index)
    npad = geom["npad"]

    use_numpy = bool(int(os.environ.get("GAT_NUMPY", "0")))
    trace = bool(int(os.environ.get("GAT_TRACE", "0")))

    wcs = [_combine_w(Ws[i], asrc[i]) for i in range(3)]
    wads = [
        np.einsum("fhc,hc->fh", Ws[i].reshape(Ws[i].shape[0], *adst[i].shape), adst[i])
        for i in range(3)
    ]
    douts = [HH * CC, HH * CC, HH * NCLS]
    outcs = [HH * CC, HH * CC, NCLS]

    progs = {}

    def run_layer(li, x_s, res_full, layer3):
        dout, outc = douts[li], outcs[li]
        wc_arr = np.ascontiguousarray(wcs[li])
        xT_arr = np.ascontiguousarray(x_s.T)
        bias_arr = np.ascontiguousarray(
            np.broadcast_to(bs[li], (P, outc)).astype(np.float32)
        )
        ad_full = (x_s @ wads[li]).astype(np.float32)
        ads = [_pack_blocks(geom, ad_full, k) for k in range(ncores)]
        ress = (
            [_pack_rows(geom, res_full, k) for k in range(ncores)]
            if not layer3
            else None
        )

        if use_numpy:
            outs = _emulate_launch(
                geom, Ma, Mb, xT_arr, wc_arr, bias_arr, idx, cofs, ads, ress,
                dout, outc, layer3,
            )
            return _unpack_rows(geom, outs)

        key = (dout, outc, layer3)
        if key not in progs:
            nc_new = _build_program(geom, Ma, Mb, cofs, dout, outc, layer3)
            nc_new.finalize()
            progs[key] = nc_new
        nc = progs[key]
        in_maps = []
        for k in range(ncores):
            im = {
                "xT": xT_arr,
                "wc": wc_arr,
                "bias": bias_arr,
                "idx": idx[k],
                "ad": ads[k],
            }
            if not layer3:
                im["res"] = ress[k]
            in_maps.append(im)
        r = run_bass_kernel_spmd(nc, in_maps, list(range(ncores)), trace=trace)
        global LAST_EXEC_NS
        if r.exec_time_ns is not None:
            LAST_EXEC_NS = (LAST_EXEC_NS or 0) + r.exec_time_ns
        outs = [np.asarray(r.results[k]["xout"]) for k in range(ncores)]
        return _unpack_rows(geom, outs)

    LAST_EXEC_NS = None
    x_s = np.zeros((npad, F), np.float32)
    x_s[:n] = x[order]

    x1 = run_layer(0, x_s, np.zeros((npad, HH * CC), np.float32), False)
    x2 = run_layer(1, x1, x1, False)
    out_s = run_layer(2, x2, None, True)

    result = np.empty((n, NCLS), np.float32)
    result[order] = out_s[:n]
    return result

